# revision 1
# baseline (speedup 1.0000x reference)
"""Trainium2 Bass kernel for BatchedActivationCSA.

Math: the reference computes, per token vector x (1024-dim):
    z   = FWHT(permute(x * signs))[:64]          (linear -> 64x1024 matrix T)
    g   = gate * z                               (per-batch gate)
    sp  = keep g_i iff |g_i| in top-16 of |g| AND |g_i| >= tau
    r   = permute^-1(FWHT(pad_64->1024(alpha*sp))) * signs   (linear -> T^T)
    out = x + r
Both linear maps are the SAME 64x1024 matrix A (FWHT is symmetric/orthonormal,
verified numerically), so the device kernel is just:
    G   = X @ A1^T        with A1 = diag(gate) @ A      [per-batch, host-built]
    SP  = topk16/tau threshold of G  (Max8 + MatchReplace + Max8 -> 16th max)
    OUT = X + SP @ A2     with A2 = alpha * A           [per-batch, host-built]
Top-16 selection == |g| >= (16th largest of |g|), exact for tie-free data.

Sharding: 8 cores, core c handles batch b=c//2, seq half c%2 -> 2048 tokens.
A1/A2/tau differ per core (per batch); same SPMD program on all cores.
"""

import numpy as np

BSZ, SEQ, DIM = 4, 4096, 1024
M = 64            # measure dim
NCORES = 8
TOK = BSZ * SEQ // NCORES      # 2048 tokens per core
TPT = 256                      # tokens per macro tile (128 partitions x 2)
NT = TOK // TPT                # 8 macro tiles per core

_cache = {}


def _fwht(y):
    """Walsh-Hadamard over last dim, identical ordering to the reference."""
    n = y.shape[-1]
    lead = y.shape[:-1]
    out = y.copy()
    h = 1
    while h < n:
        out = out.reshape(*lead, -1, 2, h)
        a, b = out[..., 0, :], out[..., 1, :]
        out = np.concatenate((a + b, a - b), axis=-1).reshape(*lead, n)
        h *= 2
    return out * (n ** -0.5)


def _build_nc():
    import concourse.bass as bass
    import concourse.mybir as mybir
    from concourse.tile import TileContext
    from concourse.masks import make_identity

    f32 = mybir.dt.float32
    f16 = mybir.dt.float16
    ACT = mybir.ActivationFunctionType
    ALU = mybir.AluOpType

    nc = bass.Bass()

    x_d = nc.dram_tensor("x", [TOK, DIM], f32, kind="ExternalInput")
    a1t_d = nc.dram_tensor("a1t", [128, 8 * M], f16, kind="ExternalInput")
    a2_d = nc.dram_tensor("a2", [M, DIM], f16, kind="ExternalInput")
    g_d = nc.dram_tensor("grep", [128, M], f32, kind="ExternalInput")
    tau_d = nc.dram_tensor("tau", [128, 1], f32, kind="ExternalInput")
    out_d = nc.dram_tensor("out", [TOK, DIM], f32, kind="ExternalOutput")

    # [2048, 1024] -> [8 tiles, 128 partitions, 2*1024]; partition p of tile t
    # holds tokens t*256+2p (cols 0:1024) and t*256+2p+1 (cols 1024:2048).
    xv = x_d[:, :].rearrange("(t p two) d -> t p (two d)", p=128, two=2)
    ov = out_d[:, :].rearrange("(t p two) d -> t p (two d)", p=128, two=2)

    with TileContext(nc) as tc:
        with (
            tc.tile_pool(name="const", bufs=1) as consts,
            tc.tile_pool(name="xin", bufs=5) as xin_pool,
            tc.tile_pool(name="xs", bufs=3) as xs_pool,
            tc.tile_pool(name="xt", bufs=4) as xt_pool,
            tc.tile_pool(name="oout", bufs=3) as out_pool,
            tc.tile_pool(name="small", bufs=6) as small,
            tc.tile_pool(name="sps", bufs=6) as sp_pool,
            tc.tile_pool(name="ps_t", bufs=2, space="PSUM") as ps_t,
            tc.tile_pool(name="ps_g", bufs=2, space="PSUM") as ps_g,
            tc.tile_pool(name="ps_s", bufs=2, space="PSUM") as ps_s,
            tc.tile_pool(name="ps_o", bufs=2, space="PSUM") as ps_o,
        ):
            a1t_s = consts.tile([128, 8 * M], f16)
            nc.sync.dma_start(a1t_s, a1t_d[:, :])
            a2_s = consts.tile([M, DIM], f16)
            nc.sync.dma_start(a2_s, a2_d[:, :])
            g_s = consts.tile([128, M], f32)
            nc.sync.dma_start(g_s, g_d[:, :])
            tau_s = consts.tile([128, 1], f32)
            nc.sync.dma_start(tau_s, tau_d[:, :])
            ident16 = consts.tile([128, 128], f16)
            make_identity(nc, ident16)
            warm = ps_t.tile([128, 512], f16, tag="pt")
            nc.tensor.transpose(warm[:, 0:128], ident16, ident16)

            def emit_cast(t, x_s):
                """split x into fp16 hi (ACT cast) + fp16 lo (GpSimd
                subtract); prefetched one tile ahead of the PE work.
                hi+lo carries 22 mantissa bits ~= fp32."""
                xh = xs_pool.tile([128, 2 * DIM], f16, tag="xh")
                xl = xs_pool.tile([128, 2 * DIM], f16, tag="xl")
                for g in range(2):
                    sl = slice(g * DIM, (g + 1) * DIM)
                    nc.scalar.activation(xh[:, sl], x_s[:, sl], ACT.Copy)
                    nc.gpsimd.tensor_tensor(
                        xl[:, sl], x_s[:, sl], xh[:, sl], ALU.subtract
                    )
                return (xh, xl)

            def emit_sense(t, x_s, xhl):
                """fp16 hi/lo transposes + 16-step mm1 (exact fp16 A) +
                topk/threshold shrink chain."""
                xh, xl = xhl
                sps = []
                for g in range(2):  # token subgroup: even / odd tokens
                    gofs = g * DIM
                    xt_s = xt_pool.tile([128, 2 * DIM], f16, tag="xt")
                    for pi, part in enumerate((xh, xl)):
                        pt = ps_t.tile([128, DIM], f16, tag="pt")
                        for c in range(8):
                            nc.tensor.transpose(
                                pt[:, c * 128:(c + 1) * 128],
                                part[:, gofs + c * 128: gofs + (c + 1) * 128],
                                ident16,
                            )
                        if pi == 0:
                            nc.scalar.activation(
                                xt_s[:, 0:DIM], pt, ACT.Copy
                            )
                        else:
                            nc.vector.tensor_copy(xt_s[:, DIM:2 * DIM], pt)
                    gp = ps_g.tile([128, M], f32, tag="g")
                    for ci in range(16):
                        nc.tensor.matmul(
                            gp,
                            lhsT=xt_s[:, ci * 128:(ci + 1) * 128],
                            rhs=a1t_s[:, (ci % 8) * M:(ci % 8 + 1) * M],
                            start=(ci == 0),
                            stop=(ci == 15),
                        )
                    az = small.tile([128, M], f32, tag="az")
                    nc.scalar.activation(az, gp, ACT.Abs)
                    z16 = small.tile([128, M], f16, tag="z16")
                    nc.vector.tensor_copy(z16, gp)
                    ag = small.tile([128, M], f32, tag="ag")
                    nc.gpsimd.tensor_tensor(ag, az, g_s, ALU.mult)
                    m8a = small.tile([128, 8], f32, tag="m8a")
                    nc.vector.max(m8a, ag)
                    agr = small.tile([128, M], f32, tag="agr")
                    nc.vector.match_replace(agr, m8a, ag, -1.0)
                    m8b = small.tile([128, 8], f32, tag="m8b")
                    nc.vector.max(m8b, agr)
                    thr = small.tile([128, 1], f32, tag="thr")
                    nc.gpsimd.tensor_single_scalar(
                        thr, m8b[:, 7:8], tau_s[:, 0:1], ALU.max
                    )
                    mask = small.tile([128, M], f32, tag="mask")
                    nc.vector.tensor_single_scalar(
                        mask, ag, thr[:, 0:1], ALU.is_ge
                    )
                    sp = sp_pool.tile([128, M], f16, tag="sp")
                    nc.vector.tensor_tensor(sp, mask, z16, ALU.mult)
                    sps.append(sp)
                return sps

            def emit_recon(t, x_s, sps):
                """sparse-transpose + mm2 + add + store for tile t."""
                o_s = out_pool.tile([128, 2 * DIM], f32, tag="o")
                for g in range(2):
                    gofs = g * DIM
                    stp = ps_s.tile([M, 128], f16, tag="st")
                    nc.tensor.transpose(stp, sps[g], ident16)
                    st_s = small.tile([M, 128], f16, tag="sts")
                    nc.scalar.activation(st_s, stp, ACT.Copy)
                    for h in range(2):
                        op = ps_o.tile([128, 512], f32, tag="op")
                        nc.tensor.matmul(
                            op,
                            lhsT=st_s,
                            rhs=a2_s[:, h * 512:(h + 1) * 512],
                            start=True,
                            stop=True,
                        )
                        nc.vector.tensor_tensor(
                            o_s[:, gofs + h * 512: gofs + (h + 1) * 512],
                            op,
                            x_s[:, gofs + h * 512: gofs + (h + 1) * 512],
                            ALU.add,
                        )
                nc.scalar.dma_start(ov[t], o_s)

            # software pipeline, 3 stages in flight:
            #   load+cast(t) | sense(t-1) | recon(t-3)
            # so the PE never waits on the ACT cast or the cross-engine
            # shrink chain (gaps downclock the PE 2.4 -> 1.2 GHz).
            casted = []
            pend = []
            for t in range(NT + 1):
                if t < NT:
                    x_s = xin_pool.tile([128, 2 * DIM], f32, tag="x")
                    nc.sync.dma_start(x_s[:, 0:DIM], xv[t][:, 0:DIM])
                    nc.sync.dma_start(x_s[:, DIM:2 * DIM], xv[t][:, DIM:2 * DIM])
                    xhl = emit_cast(t, x_s)
                    casted.append((t, x_s, xhl))
                if casted and (t >= 1):
                    ct, cx_s, cxhl = casted.pop(0)
                    sps = emit_sense(ct, cx_s, cxhl)
                    pend.append((ct, cx_s, sps))
                while len(pend) > 1:
                    emit_recon(*pend.pop(0))
            for args in pend:
                emit_recon(*args)

    _split_pe_waits(nc, mybir)
    return nc


def _split_pe_waits(nc, mybir):
    """walrus codegen allows only one sync wait on most compute instruction
    structs (PE LDWEIGHTS, DVE TS, ...). Move the waits of any multi-wait
    compute instruction onto a NoOp inserted just before it: each engine's
    sequencer executes in order, so all waits still happen-before it."""
    skip = (
        mybir.InstNoOp,
        mybir.InstEventSemaphore,
        mybir.InstUnconditionalBranch,
        mybir.InstRegisterMove,
    )
    for f in nc.m.functions:
        for blk in f.blocks:
            insts = list(blk.instructions)
            out = []
            changed = False
            for ins in insts:
                si = getattr(ins, "sync_info", None)
                if (
                    not isinstance(ins, skip)
                    and getattr(ins, "engine", None) is not None
                    and si is not None
                    and si.on_wait
                    and len(si.on_wait) > 1
                ):
                    waits = list(si.on_wait)
                    for k, w in enumerate(waits[:-1]):
                        nop = mybir.InstNoOp(
                            name=f"{ins.name}-waitsplit{k}", ins=[], outs=[]
                        )
                        nop.engine = ins.engine
                        nop.sync_info = mybir.SyncInfo(
                            on_wait=[w], on_update=[]
                        )
                        out.append(nop)
                    ins.sync_info = mybir.SyncInfo(
                        on_wait=[waits[-1]], on_update=list(si.on_update)
                    )
                    changed = True
                out.append(ins)
            if changed:
                blk.instructions = out


def _prep_inputs(x, gates, alpha, tau, signs, perm, inv_perm, target_idx):
    """Host-side prep: build per-core input maps (small matrices only)."""
    tidx = int(target_idx)
    signs = np.asarray(signs, dtype=np.float64)
    perm = np.asarray(perm, dtype=np.int64)
    inv_perm = np.asarray(inv_perm, dtype=np.int64)

    # Sense matrix A: row i = i-th output of FWHT(permute(e * signs))[:64].
    eye = np.eye(DIM, dtype=np.float64)
    A = _fwht((eye * signs[None, :])[:, perm])[:, :M].T          # [64, 1024]
    # Reconstruct matrix B (provably == A, but built independently for safety)
    pad = np.zeros((M, DIM), dtype=np.float64)
    pad[:, :M] = np.eye(M)
    B = _fwht(pad)[:, inv_perm] * signs[None, :]                 # [64, 1024]

    in_maps = []
    for c in range(NCORES):
        b, half = divmod(c, 2)
        g = np.asarray(gates, dtype=np.float64)[b, tidx]         # [64]
        al = float(np.asarray(alpha, dtype=np.float64)[b, tidx, 0])
        tu = abs(float(np.asarray(tau, dtype=np.float64)[b, tidx, 0]))
        # a1t: exact +-1/32 A^T (ungated; fp16-exact). The gate is applied
        # on-device to |z| for ranking, and folded into a2 for values.
        a1t = np.ascontiguousarray(
            A.T.reshape(8, 128, M).transpose(1, 0, 2).reshape(128, 8 * M)
        ).astype(np.float16)
        A2 = (al * g[:, None] * B).astype(np.float16)            # [64, 1024]
        xs = np.ascontiguousarray(
            np.asarray(x)[b, half * TOK:(half + 1) * TOK, :], dtype=np.float32
        )
        in_maps.append({
            "x": xs,
            "a1t": a1t,
            "a2": np.ascontiguousarray(A2),
            "grep": np.broadcast_to(g.astype(np.float32), (128, M)).copy(),
            "tau": np.full((128, 1), tu, dtype=np.float32),
        })
    return in_maps


def _get_nc():
    if "nc" not in _cache:
        _cache["nc"] = _build_nc()
    return _cache["nc"]


def kernel(x, gates, alpha, tau, signs, perm, inv_perm, target_idx,
           _trace=False, _tmpdir=None):
    from concourse.bass_utils import run_bass_kernel_spmd

    nc = _get_nc()
    in_maps = _prep_inputs(x, gates, alpha, tau, signs, perm, inv_perm,
                           target_idx)
    res = run_bass_kernel_spmd(
        nc, in_maps, core_ids=list(range(NCORES)),
        trace=_trace, tmpdir=_tmpdir,
    )
    if _trace:
        _cache["last_results"] = res
    out = np.empty((BSZ, SEQ, DIM), dtype=np.float32)
    for c in range(NCORES):
        b, half = divmod(c, 2)
        out[b, half * TOK:(half + 1) * TOK, :] = res.results[c]["out"]
    return out



# revision 4
# speedup vs baseline: 1.7078x; 1.7078x over previous
"""Trainium2 Bass kernel for BatchedActivationCSA.

Math: per token vector x (1024-dim):
    z   = FWHT(permute(x * signs))[:64]          (linear -> 64x1024 matrix A)
    g   = gate * z
    sp  = keep g_i iff |g_i| in top-16 of |g| AND |g_i| >= tau
    r   = permute^-1(FWHT(pad_64->1024(alpha*sp))) * signs   (linear -> A^T)
    out = x + r

Device kernel (per core, 2048 tokens):
    G   = X @ A1^T   with A1 = diag(gate) @ A    [gated values, ranking = |G|]
    SP  = topk16/tau threshold of G  (Max8 + MatchReplace + Max8 -> 16th max)
    OUT^T = X^T + A2^T-chunks @ SP^T   with A2 = alpha * A

Layout: everything d-major ("transposed") on device. The host uploads
x^T as fp16 so the PE needs NO input transposes and HBM traffic is half
of fp32. Output comes back as out^T fp16. Host does the (un)transpose +
casts during shard/unshard. Precision: fp16 end-to-end gives ~1e-3 max
rel err (gate is 2e-2).

Sharding: 8 cores, core c handles batch b=c//2, seq half c%2 -> 2048 tokens.
"""

import numpy as np

BSZ, SEQ, DIM = 4, 4096, 1024
M = 64            # measure dim
NCORES = 8
TOK = BSZ * SEQ // NCORES      # 2048 tokens per core
BLK = 512                      # tokens per block
NB = TOK // BLK                # 4 blocks
NC_ = 8                        # d-chunks of 128

_cache = {}


def _fwht(y):
    """Walsh-Hadamard over last dim, identical ordering to the reference."""
    n = y.shape[-1]
    lead = y.shape[:-1]
    out = y.copy()
    h = 1
    while h < n:
        out = out.reshape(*lead, -1, 2, h)
        a, b = out[..., 0, :], out[..., 1, :]
        out = np.concatenate((a + b, a - b), axis=-1).reshape(*lead, n)
        h *= 2
    return out * (n ** -0.5)


def _build_nc():
    import concourse.bass as bass
    import concourse.mybir as mybir
    from concourse.tile import TileContext
    from concourse.masks import make_identity

    f32 = mybir.dt.float32
    f16 = mybir.dt.float16
    ACT = mybir.ActivationFunctionType
    ALU = mybir.AluOpType

    nc = bass.Bass()

    xt_d = nc.dram_tensor("xt", [NB * 128, NC_ * BLK], f16, kind="ExternalInput")
    a1t_d = nc.dram_tensor("a1t", [128, NC_ * M], f16, kind="ExternalInput")
    a2_d = nc.dram_tensor("a2", [M, DIM], f16, kind="ExternalInput")
    tau_d = nc.dram_tensor("tau", [128, 1], f32, kind="ExternalInput")
    od_d = nc.dram_tensor("od", [NB * 128, NC_ * BLK], f16, kind="ExternalOutput")

    xv = xt_d[:, :].rearrange("(b p) f -> b p f", p=128)
    ov = od_d[:, :].rearrange("(b p) f -> b p f", p=128)

    with TileContext(nc) as tc:
        with (
            tc.tile_pool(name="const", bufs=1) as consts,
            tc.tile_pool(name="xin", bufs=NB) as xin_pool,
            tc.tile_pool(name="oout", bufs=2) as out_pool,
            tc.tile_pool(name="st", bufs=2) as st_pool,
            tc.tile_pool(name="small", bufs=8) as small,
            tc.tile_pool(name="ps_g", bufs=4, space="PSUM") as ps_g,
            tc.tile_pool(name="ps_t", bufs=2, space="PSUM") as ps_t,
            tc.tile_pool(name="ps_o", bufs=2, space="PSUM") as ps_o,
        ):
            a1t_s = consts.tile([128, NC_ * M], f16)
            nc.sync.dma_start(a1t_s, a1t_d[:, :])
            a2_s = consts.tile([M, DIM], f16)
            nc.sync.dma_start(a2_s, a2_d[:, :])
            tau_s = consts.tile([128, 1], f32)
            nc.sync.dma_start(tau_s, tau_d[:, :])
            ident16 = consts.tile([128, 128], f16)
            make_identity(nc, ident16)
            warm = ps_t.tile([64, 128], f16, tag="pt")
            nc.tensor.transpose(warm, ident16[0:128, 0:64], ident16)

            # stream all input blocks up front; they drain at line rate
            xts = []
            for b in range(NB):
                xt_s = xin_pool.tile([128, NC_ * BLK], f16, tag="x")
                nc.sync.dma_start(xt_s, xv[b])
                xts.append(xt_s)

            for b in range(NB):
                xt_s = xts[b]
                st4 = st_pool.tile([M, BLK], f16, tag="st4")
                for g4 in range(BLK // 128):
                    gofs = g4 * 128
                    gp = ps_g.tile([128, M], f32, tag="g")
                    for c in range(NC_):
                        nc.tensor.matmul(
                            gp,
                            lhsT=xt_s[:, c * BLK + gofs: c * BLK + gofs + 128],
                            rhs=a1t_s[:, c * M:(c + 1) * M],
                            start=(c == 0),
                            stop=(c == NC_ - 1),
                        )
                    ag = small.tile([128, M], f32, tag="ag")
                    nc.scalar.activation(ag, gp, ACT.Abs)
                    m8a = small.tile([128, 8], f32, tag="m8a")
                    nc.vector.max(m8a, ag)
                    agr = small.tile([128, M], f32, tag="agr")
                    nc.vector.match_replace(agr, m8a, ag, -1.0)
                    m8b = small.tile([128, 8], f32, tag="m8b")
                    nc.vector.max(m8b, agr)
                    thr = small.tile([128, 1], f32, tag="thr")
                    nc.gpsimd.tensor_single_scalar(
                        thr, m8b[:, 7:8], tau_s[:, 0:1], ALU.max
                    )
                    # sp = (|G| >= thr) * G, fp16
                    sp = small.tile([128, M], f16, tag="sp")
                    nc.vector.scalar_tensor_tensor(
                        sp, ag, thr[:, 0:1], gp, ALU.is_ge, ALU.mult
                    )
                    stp = ps_t.tile([M, 128], f16, tag="pt")
                    nc.tensor.transpose(stp, sp, ident16)
                    nc.scalar.activation(
                        st4[:, gofs:gofs + 128], stp, ACT.Copy
                    )
                ot = out_pool.tile([128, NC_ * BLK], f16, tag="o")
                for c in range(NC_):
                    op = ps_o.tile([128, BLK], f32, tag="op")
                    nc.tensor.matmul(
                        op,
                        lhsT=a2_s[:, c * 128:(c + 1) * 128],
                        rhs=st4,
                        start=True,
                        stop=True,
                    )
                    # GpSimd can't touch PSUM: route 5/8 adds through DVE
                    # directly; for the rest ACT drains PSUM->SBUF fp16 and
                    # GpSimd does the SBUF-only add.
                    if c % 3 != 2:
                        nc.vector.tensor_tensor(
                            ot[:, c * BLK:(c + 1) * BLK],
                            op,
                            xt_s[:, c * BLK:(c + 1) * BLK],
                            ALU.add,
                        )
                    else:
                        rtmp = small.tile([128, BLK], f16, tag="rt")
                        nc.scalar.activation(rtmp, op, ACT.Copy)
                        nc.gpsimd.tensor_tensor(
                            ot[:, c * BLK:(c + 1) * BLK],
                            rtmp,
                            xt_s[:, c * BLK:(c + 1) * BLK],
                            ALU.add,
                        )
                nc.scalar.dma_start(ov[b], ot)

    _split_multi_waits(nc, mybir)
    return nc


def _split_multi_waits(nc, mybir):
    """walrus codegen allows only one sync wait on most compute instruction
    structs (PE LDWEIGHTS, DVE TS, ...). Move the waits of any multi-wait
    compute instruction onto a NoOp inserted just before it: each engine's
    sequencer executes in order, so all waits still happen-before it."""
    skip = (
        mybir.InstNoOp,
        mybir.InstEventSemaphore,
        mybir.InstUnconditionalBranch,
        mybir.InstRegisterMove,
    )
    for f in nc.m.functions:
        for blk in f.blocks:
            insts = list(blk.instructions)
            out = []
            changed = False
            for ins in insts:
                si = getattr(ins, "sync_info", None)
                if (
                    not isinstance(ins, skip)
                    and getattr(ins, "engine", None) is not None
                    and si is not None
                    and si.on_wait
                    and len(si.on_wait) > 1
                ):
                    waits = list(si.on_wait)
                    for k, w in enumerate(waits[:-1]):
                        nop = mybir.InstNoOp(
                            name=f"{ins.name}-waitsplit{k}", ins=[], outs=[]
                        )
                        nop.engine = ins.engine
                        nop.sync_info = mybir.SyncInfo(
                            on_wait=[w], on_update=[]
                        )
                        out.append(nop)
                    ins.sync_info = mybir.SyncInfo(
                        on_wait=[waits[-1]], on_update=list(si.on_update)
                    )
                    changed = True
                out.append(ins)
            if changed:
                blk.instructions = out


def _prep_inputs(x, gates, alpha, tau, signs, perm, inv_perm, target_idx):
    """Host-side prep: shard + transpose + cast per core."""
    tidx = int(target_idx)
    signs = np.asarray(signs, dtype=np.float64)
    perm = np.asarray(perm, dtype=np.int64)
    inv_perm = np.asarray(inv_perm, dtype=np.int64)
    x = np.asarray(x)

    # Sense matrix A: row i = i-th output of FWHT(permute(e * signs))[:64].
    eye = np.eye(DIM, dtype=np.float64)
    A = _fwht((eye * signs[None, :])[:, perm])[:, :M].T          # [64, 1024]
    # Reconstruct matrix B (== A, but built independently for safety)
    pad = np.zeros((M, DIM), dtype=np.float64)
    pad[:, :M] = np.eye(M)
    B = _fwht(pad)[:, inv_perm] * signs[None, :]                 # [64, 1024]

    in_maps = []
    for c in range(NCORES):
        b, half = divmod(c, 2)
        g = np.asarray(gates, dtype=np.float64)[b, tidx]         # [64]
        al = float(np.asarray(alpha, dtype=np.float64)[b, tidx, 0])
        tu = abs(float(np.asarray(tau, dtype=np.float64)[b, tidx, 0]))
        a1 = g[:, None] * A                                      # [64, 1024]
        a1t = np.ascontiguousarray(
            a1.T.reshape(NC_, 128, M).transpose(1, 0, 2).reshape(128, NC_ * M)
        ).astype(np.float16)
        a2 = (al * B).astype(np.float16)                         # [64, 1024]
        xs = x[b, half * TOK:(half + 1) * TOK, :].astype(np.float16)
        # [tok, dim] -> [blk, p, c, t] -> [NB*128, NC_*BLK]
        xt = np.ascontiguousarray(
            xs.reshape(NB, BLK, NC_, 128).transpose(0, 3, 2, 1)
        ).reshape(NB * 128, NC_ * BLK)
        in_maps.append({
            "xt": xt,
            "a1t": a1t,
            "a2": np.ascontiguousarray(a2),
            "tau": np.full((128, 1), tu, dtype=np.float32),
        })
    return in_maps


def _get_nc():
    if "nc" not in _cache:
        _cache["nc"] = _build_nc()
    return _cache["nc"]


def kernel(x, gates, alpha, tau, signs, perm, inv_perm, target_idx,
           _trace=False, _tmpdir=None):
    from concourse.bass_utils import run_bass_kernel_spmd

    nc = _get_nc()
    in_maps = _prep_inputs(x, gates, alpha, tau, signs, perm, inv_perm,
                           target_idx)
    res = run_bass_kernel_spmd(
        nc, in_maps, core_ids=list(range(NCORES)),
        trace=_trace, tmpdir=_tmpdir,
    )
    if _trace:
        _cache["last_results"] = res
    out = np.empty((BSZ, SEQ, DIM), dtype=np.float32)
    for c in range(NCORES):
        b, half = divmod(c, 2)
        od = res.results[c]["od"]
        # [NB*128, NC_*BLK] -> [blk, p, c, t] -> [tok, dim]
        o = od.reshape(NB, 128, NC_, BLK).transpose(0, 3, 2, 1).reshape(
            TOK, DIM)
        out[b, half * TOK:(half + 1) * TOK, :] = o.astype(np.float32)
    return out
